# revision 4
# baseline (speedup 1.0000x reference)
"""Trainium2 Bass kernel for DGS3DLayer: 3D trilinear grid-sample with analytic
gradients (phi, d/dx, d/dy, d/dz), f32.

Strategy (8 NeuronCores):
  - shard over (batch, query): core i handles batch i//4, query quarter i%4
    (25000 queries padded to 25600 = 128 partitions x 200).
  - host prep (data movement only): volume -> channel-last -> redundant
    "V4" layout where row r=(z0,y0,x0) holds the 2x2x2x16ch corner block
    (512B). Every query then needs exactly ONE contiguous 512B chunk.
  - device: compute voxel indices + fractions from the grid on DVE, gather
    chunks with [P,1]-offset indirect DMAs (128 queries per instruction),
    do the factorized trilinear interpolation + analytic derivatives on DVE,
    write (16ch,4out,q) staging and a single strided DMA to DRAM.
  - host: concatenate per-core outputs, drop padding.
"""
import numpy as np

import concourse.bass as bass
import concourse.bacc as bacc
import concourse.mybir as mybir
import concourse.tile as tile

B, C, D, H, W = 2, 16, 128, 128, 128
Q = 100000
QCORE = 25000          # real queries per core
QC = 25600             # padded queries per core (128 x 200)
QP = 200               # queries per partition
K = 40                 # queries-per-partition per compute group
NG = QP // K           # 5 groups
N4 = 127 * 127 * 127   # V4 rows
P = 128

f32 = mybir.dt.float32
i32 = mybir.dt.int32
OP = mybir.AluOpType

_CACHE = {}


def _build_nc(sx, sy, sz):
    """Build + compile the SPMD Bass program. sx/sy/sz = (dim-1)/size scales."""
    nc = bacc.Bacc("TRN2", target_bir_lowering=False, debug=False, num_devices=8)

    t_v4 = nc.dram_tensor("v4", [N4, 128], f32, kind="ExternalInput")
    t_grid = nc.dram_tensor("gridT", [3, QC], f32, kind="ExternalInput")
    t_out = nc.dram_tensor("out", [C, 4, QC], f32, kind="ExternalOutput")

    with tile.TileContext(nc) as tc:
        with tc.tile_pool(name="cpool", bufs=1) as cp:
            # ---- load grid (x,y,z rows are contiguous length-QC vectors) ----
            gx = cp.tile([P, QP], f32)
            gy = cp.tile([P, QP], f32)
            gz = cp.tile([P, QP], f32)
            nc.sync.dma_start(out=gx[:], in_=t_grid[0].rearrange("(p k) -> p k", p=P))
            nc.sync.dma_start(out=gy[:], in_=t_grid[1].rearrange("(p k) -> p k", p=P))
            nc.sync.dma_start(out=gz[:], in_=t_grid[2].rearrange("(p k) -> p k", p=P))

            # ---- per-axis: f = (g+1)*0.5*(dim-1); i0 = clip(floor(f),0,dim-2); t = f-i0
            def axis_prep(g, dim, ax):
                f = cp.tile([P, QP], f32, name=f"f_{ax}", tag=f"f_{ax}")
                nc.vector.tensor_scalar(
                    out=f[:], in0=g[:],
                    scalar1=0.5 * (dim - 1), scalar2=0.5 * (dim - 1),
                    op0=OP.mult, op1=OP.add,
                )
                ii = cp.tile([P, QP], i32, name=f"ii_{ax}", tag=f"ii_{ax}")
                nc.vector.tensor_copy(out=ii[:], in_=f[:])       # may round
                i0f = cp.tile([P, QP], f32, name=f"i0f_{ax}", tag=f"i0f_{ax}")
                nc.vector.tensor_copy(out=i0f[:], in_=ii[:])
                fix = cp.tile([P, QP], f32, name=f"fix_{ax}", tag=f"fix_{ax}")
                nc.vector.tensor_tensor(out=fix[:], in0=i0f[:], in1=f[:], op=OP.is_gt)
                nc.vector.tensor_tensor(out=i0f[:], in0=i0f[:], in1=fix[:], op=OP.subtract)
                nc.vector.tensor_scalar_min(i0f[:], i0f[:], float(dim - 2))
                nc.vector.tensor_scalar_max(i0f[:], i0f[:], 0.0)
                t = cp.tile([P, QP], f32, name=f"t_{ax}", tag=f"t_{ax}")
                nc.vector.tensor_tensor(out=t[:], in0=f[:], in1=i0f[:], op=OP.subtract)
                return i0f, t

            x0f, tx = axis_prep(gx, W, "x")
            y0f, ty = axis_prep(gy, H, "y")
            z0f, tz = axis_prep(gz, D, "z")

            tz_s = cp.tile([P, QP], f32)   # tz * sz (pre-scaled for grad folds)
            nc.vector.tensor_scalar_mul(out=tz_s[:], in0=tz[:], scalar1=float(sz))

            # ---- linear V4 row index = (z0*127 + y0)*127 + x0 (exact in f32) ----
            idxf = cp.tile([P, QP], f32)
            nc.vector.scalar_tensor_tensor(
                out=idxf[:], in0=z0f[:], scalar=127.0, in1=y0f[:],
                op0=OP.mult, op1=OP.add)
            nc.vector.scalar_tensor_tensor(
                out=idxf[:], in0=idxf[:], scalar=127.0, in1=x0f[:],
                op0=OP.mult, op1=OP.add)
            idx = cp.tile([P, QP], i32)
            nc.vector.tensor_copy(out=idx[:], in_=idxf[:])

            # ---- output staging: [p, (ch, o, k)] ----
            stag = cp.tile([P, C * 4 * QP], f32)
            stag_v = stag[:].rearrange("p (c o k) -> p o k c", c=C, o=4, k=QP)

            with tc.tile_pool(name="gpool", bufs=2) as gp, \
                 tc.tile_pool(name="wpool", bufs=1) as wp:
                for g in range(NG):
                    sl = slice(g * K, (g + 1) * K)
                    gt = gp.tile([P, K * 128], f32, tag="gt")
                    for kk in range(K):
                        j = g * K + kk
                        nc.gpsimd.indirect_dma_start(
                            out=gt[:, kk * 128:(kk + 1) * 128],
                            out_offset=None,
                            in_=t_v4[:],
                            in_offset=bass.IndirectOffsetOnAxis(
                                ap=idx[:, j:j + 1], axis=0),
                        )

                    # views of the gathered chunk: [p, k, zy, x, c]  (zy = z*2+y)
                    gv = gt[:].rearrange(
                        "p (k zy x c) -> p k zy x c", k=K, zy=4, x=2, c=C)
                    A0 = gv[:, :, :, 0, :]   # [p,k,4,16] — 3 free dims
                    A1 = gv[:, :, :, 1, :]

                    def bc(t2, shape):
                        a = t2[:, sl]
                        for _ in range(len(shape) - 2):
                            a = a.unsqueeze(2)
                        return a.to_broadcast(shape)

                    txb = bc(tx, [P, K, 4, C])
                    tyb3 = bc(ty, [P, K, 2, C])
                    tzb3 = bc(tz, [P, K, C])
                    tzsb3 = bc(tz_s, [P, K, C])

                    # x-lerp: D4 = A1-A0 ; CX = A0 + D4*tx     [p,k,zy4,16]
                    D4 = wp.tile([P, K * 4 * C], f32, tag="D4")
                    D4m = D4[:].rearrange("p (k zy c) -> p k zy c", k=K, zy=4, c=C)
                    D4v = D4[:].rearrange("p (k z y c) -> p k z y c", k=K, z=2, y=2, c=C)
                    nc.vector.tensor_tensor(out=D4m, in0=A1, in1=A0, op=OP.subtract)
                    T1 = wp.tile([P, K * 4 * C], f32, tag="T1")
                    T1m = T1[:].rearrange("p (k zy c) -> p k zy c", k=K, zy=4, c=C)
                    nc.vector.tensor_tensor(out=T1m, in0=D4m, in1=txb, op=OP.mult)
                    CX = wp.tile([P, K * 4 * C], f32, tag="CX")
                    CXm = CX[:].rearrange("p (k zy c) -> p k zy c", k=K, zy=4, c=C)
                    CXv = CX[:].rearrange("p (k z y c) -> p k z y c", k=K, z=2, y=2, c=C)
                    nc.vector.tensor_tensor(out=CXm, in0=A0, in1=T1m, op=OP.add)

                    # y-lerp: DY = CX[y1]-CX[y0]; CY = CX[y0] + DY*ty   [p,k,2z,16]
                    DY = wp.tile([P, K * 2 * C], f32, tag="DY")
                    DYv = DY[:].rearrange("p (k z c) -> p k z c", k=K, z=2, c=C)
                    nc.vector.tensor_tensor(
                        out=DYv, in0=CXv[:, :, :, 1, :], in1=CXv[:, :, :, 0, :],
                        op=OP.subtract)
                    T2 = wp.tile([P, K * 2 * C], f32, tag="T2")
                    T2v = T2[:].rearrange("p (k z c) -> p k z c", k=K, z=2, c=C)
                    nc.vector.tensor_tensor(out=T2v, in0=DYv, in1=tyb3, op=OP.mult)
                    CY = wp.tile([P, K * 2 * C], f32, tag="CY")
                    CYv = CY[:].rearrange("p (k z c) -> p k z c", k=K, z=2, c=C)
                    nc.vector.tensor_tensor(
                        out=CYv, in0=CXv[:, :, :, 0, :], in1=T2v, op=OP.add)

                    # z-lerp: DZ = CY[z1]-CY[z0]; phi = CY[z0] + DZ*tz; gz = DZ*sz
                    DZ = wp.tile([P, K * C], f32, tag="DZ")
                    DZv = DZ[:].rearrange("p (k c) -> p k c", k=K, c=C)
                    nc.vector.tensor_tensor(
                        out=DZv, in0=CYv[:, :, 1, :], in1=CYv[:, :, 0, :],
                        op=OP.subtract)
                    T3 = wp.tile([P, K * C], f32, tag="T3")
                    T3v = T3[:].rearrange("p (k c) -> p k c", k=K, c=C)
                    nc.vector.tensor_tensor(out=T3v, in0=DZv, in1=tzb3, op=OP.mult)
                    nc.vector.tensor_tensor(
                        out=stag_v[:, 0, sl, :], in0=CYv[:, :, 0, :], in1=T3v,
                        op=OP.add)
                    nc.scalar.mul(out=stag_v[:, 3, sl, :], in_=DZv, mul=float(sz))

                    # ddy = DY[z0] + (DY[z1]-DY[z0])*tz ; gy = ddy*sy
                    E = wp.tile([P, K * C], f32, tag="E")
                    Ev = E[:].rearrange("p (k c) -> p k c", k=K, c=C)
                    nc.vector.tensor_tensor(
                        out=Ev, in0=DYv[:, :, 1, :], in1=DYv[:, :, 0, :],
                        op=OP.subtract)
                    T4 = wp.tile([P, K * C], f32, tag="T4")
                    T4v = T4[:].rearrange("p (k c) -> p k c", k=K, c=C)
                    # note sy == sz here in general? no: use tz*sy = tz_s*(sy/sz) — keep
                    # separate: T4 = E * tz, then STT fold with sy.
                    nc.vector.tensor_tensor(out=T4v, in0=Ev, in1=tzb3, op=OP.mult)
                    nc.vector.scalar_tensor_tensor(
                        out=stag_v[:, 2, sl, :], in0=DYv[:, :, 0, :],
                        scalar=1.0, in1=T4v, op0=OP.mult, op1=OP.add,
                    )
                    nc.scalar.mul(
                        out=stag_v[:, 2, sl, :], in_=stag_v[:, 2, sl, :],
                        mul=float(sy))

                    # ddx: Ey[z] = D4[y0] + (D4[y1]-D4[y0])*ty ;
                    # gx = (Ey[z0] + (Ey[z1]-Ey[z0])*tz) * sx
                    F = wp.tile([P, K * 2 * C], f32, tag="F")
                    Fv = F[:].rearrange("p (k z c) -> p k z c", k=K, z=2, c=C)
                    nc.vector.tensor_tensor(
                        out=Fv, in0=D4v[:, :, :, 1, :], in1=D4v[:, :, :, 0, :],
                        op=OP.subtract)
                    T5 = wp.tile([P, K * 2 * C], f32, tag="T5")
                    T5v = T5[:].rearrange("p (k z c) -> p k z c", k=K, z=2, c=C)
                    nc.vector.tensor_tensor(out=T5v, in0=Fv, in1=tyb3, op=OP.mult)
                    Ey = wp.tile([P, K * 2 * C], f32, tag="Ey")
                    Eyv = Ey[:].rearrange("p (k z c) -> p k z c", k=K, z=2, c=C)
                    nc.vector.tensor_tensor(
                        out=Eyv, in0=D4v[:, :, :, 0, :], in1=T5v, op=OP.add)
                    G2 = wp.tile([P, K * C], f32, tag="G2")
                    G2v = G2[:].rearrange("p (k c) -> p k c", k=K, c=C)
                    nc.vector.tensor_tensor(
                        out=G2v, in0=Eyv[:, :, 1, :], in1=Eyv[:, :, 0, :],
                        op=OP.subtract)
                    T6 = wp.tile([P, K * C], f32, tag="T6")
                    T6v = T6[:].rearrange("p (k c) -> p k c", k=K, c=C)
                    nc.vector.tensor_tensor(out=T6v, in0=G2v, in1=tzsb3, op=OP.mult)
                    nc.vector.scalar_tensor_tensor(
                        out=stag_v[:, 1, sl, :], in0=Eyv[:, :, 0, :],
                        scalar=float(sz), in1=T6v, op0=OP.mult, op1=OP.add,
                    )
                    if abs(sx - sz) > 1e-12:
                        nc.scalar.mul(
                            out=stag_v[:, 1, sl, :], in_=stag_v[:, 1, sl, :],
                            mul=float(sx / sz))

            # ---- one big strided output DMA: stag [p,(c,o,k)] -> out[c,o,p*200+k]
            out_ap = t_out[:].rearrange("c o (p k) -> p c o k", p=P)
            nc.sync.dma_start(out=out_ap, in_=stag[:].rearrange(
                "p (c o k) -> p c o k", c=C, o=4, k=QP))

    nc.compile()
    return nc


class _Runner:
    """Compiled-once SPMD executor over 8 axon NeuronCores (PJRT shard_map)."""

    def __init__(self, nc):
        import jax
        from jax.sharding import Mesh, PartitionSpec
        from jax.experimental.shard_map import shard_map
        from concourse import bass2jax

        bass2jax.install_neuronx_cc_hook()
        self.jax = jax
        self.nc = nc
        n = 8
        partition_name = nc.partition_id_tensor.name if nc.partition_id_tensor else None
        in_names, out_names, out_avals = [], [], []
        for alloc in nc.m.functions[0].allocations:
            if not isinstance(alloc, mybir.MemoryLocationSet):
                continue
            name = alloc.memorylocations[0].name
            if alloc.kind == "ExternalInput":
                if name != partition_name:
                    in_names.append(name)
            elif alloc.kind == "ExternalOutput":
                out_names.append(name)
                out_avals.append(jax.core.ShapedArray(
                    tuple(alloc.tensor_shape), mybir.dt.np(alloc.dtype)))
        self.in_names, self.out_names, self.out_avals = in_names, out_names, out_avals
        n_params, n_outs = len(in_names), len(out_avals)
        all_in = in_names + out_names + ([partition_name] if partition_name else [])

        def _body(*args):
            operands = list(args)
            if partition_name is not None:
                operands.append(bass2jax.partition_id_tensor())
            return tuple(bass2jax._bass_exec_p.bind(
                *operands, out_avals=tuple(out_avals), in_names=tuple(all_in),
                out_names=tuple(out_names), lowering_input_output_aliases=(),
                sim_require_finite=True, sim_require_nnan=True, nc=nc))

        devices = jax.devices()[:n]
        self.mesh = Mesh(np.asarray(devices), ("core",))
        self.spec = PartitionSpec("core")
        self.sharded = jax.jit(
            shard_map(_body, mesh=self.mesh,
                      in_specs=(self.spec,) * (n_params + n_outs),
                      out_specs=(self.spec,) * n_outs, check_rep=False),
            donate_argnums=tuple(range(n_params, n_params + n_outs)),
            keep_unused=True)

    def run(self, in_maps):
        jax = self.jax
        sh = jax.sharding.NamedSharding(self.mesh, self.spec)
        dev_in = [
            jax.device_put(
                np.concatenate([np.asarray(in_maps[c][name]) for c in range(8)],
                               axis=0), sh)
            for name in self.in_names
        ]
        zouts = [
            jax.device_put(np.zeros((8 * a.shape[0], *a.shape[1:]), a.dtype), sh)
            for a in self.out_avals
        ]
        outs = self.sharded(*dev_in, *zouts)
        jax.block_until_ready(outs)
        return [
            {name: np.asarray(outs[i]).reshape(8, *self.out_avals[i].shape)[c]
             for i, name in enumerate(self.out_names)}
            for c in range(8)
        ]


def _prep_v4(vol_b):
    """vol_b: np [C, D, H, W] -> V4 [127^3, 128] f32 (pure layout transform)."""
    cl = np.ascontiguousarray(np.transpose(vol_b, (1, 2, 3, 0)))  # [D,H,W,C]
    sw = np.lib.stride_tricks.sliding_window_view(cl, (2, 2, 2), axis=(0, 1, 2))
    # sw: [127,127,127, C, 2,2,2] -> want [127,127,127, 2,2,2, C]
    v4 = np.ascontiguousarray(np.transpose(sw, (0, 1, 2, 4, 5, 6, 3)))
    return v4.reshape(N4, 128)


def kernel(input, grid, sizeX, sizeY, sizeZ):
    input = np.asarray(input, dtype=np.float32)
    grid = np.asarray(grid, dtype=np.float32)
    sx = (W - 1) / float(sizeX)
    sy = (H - 1) / float(sizeY)
    sz = (D - 1) / float(sizeZ)

    key = ("nc", round(sx, 9), round(sy, 9), round(sz, 9))
    if key not in _CACHE:
        nc = _build_nc(sx, sy, sz)
        _CACHE[key] = _Runner(nc)
    runner = _CACHE[key]

    v4s = [_prep_v4(input[b]) for b in range(B)]
    in_maps = []
    for core in range(8):
        b, qpart = core // 4, core % 4
        g = grid[b, qpart * QCORE:(qpart + 1) * QCORE]      # [25000, 3]
        gt = np.zeros((3, QC), np.float32)
        gt[:, :QCORE] = g.T
        in_maps.append({"v4": v4s[b], "gridT": gt})

    res = runner.run(in_maps)

    out = np.empty((B, C, 4, Q), np.float32)
    for core in range(8):
        b, qpart = core // 4, core % 4
        out[b, :, :, qpart * QCORE:(qpart + 1) * QCORE] = \
            res[core]["out"][:, :, :QCORE]
    return out
